# revision 5
# baseline (speedup 1.0000x reference)
"""ChunkDropout forward on 8 Trainium2 NeuronCores.

out = x * (1 - mask) * scaling_factor, where mask is the deterministic
chunk-dropout mask drawn from jax.random.key(42) (length-L boolean).

Strategy: pure data parallel. The mask depends only on a hardcoded RNG key,
so it is computed once on host CPU; (1 - mask) * scaling_factor folds into a
single [1, L] f32 vector replicated to every core. x is sharded 4096 -> 8 x
512 rows. Per core the Bass kernel broadcasts the scale vector across the
128 SBUF partitions once, then streams [128, 2500] tiles: DMA load ->
in-place tensor_mul on the vector engine -> DMA store. Memory-bound; roughly
41 MB of HBM traffic per core.
"""

import numpy as np

BATCH = 4096
L = 10000
N_CORES = 8
ROWS = BATCH // N_CORES  # 512 rows per core
P = 128                  # SBUF partitions
RB = ROWS // P           # 4 row blocks per core
FCH = 2500               # free-dim chunk
NF = L // FCH            # 4 chunks

# Mask hyperparameters (must match the reference module exactly)
DROPOUT_P = 0.01
HOLE_LOC = 10.0
HOLE_SCALE = 3.0
MIN_HOLE = 1

_cache = {}


def _chunk_mask_np():
    """The chunk-dropout mask for jax.random.key(42), computed on host CPU.

    Faithful translation of the reference sampler: geometric gap offsets and
    truncated/clamped normal hole lengths, sequentially OR-ed into a length-L
    boolean mask. Runs eagerly on CPU jax so device placement never touches
    the TRN backend.
    """
    if "mask" in _cache:
        return _cache["mask"]
    import jax
    import jax.numpy as jnp

    with jax.default_device(jax.devices("cpu")[0]):
        key = jax.random.key(42)
        log1mp = float(np.log(np.float32(1.0 - DROPOUT_P)))
        mask = np.zeros(L, dtype=bool)
        last_end = 0
        while True:
            key, kg, kn = jax.random.split(key, 3)
            u = float(
                jax.random.uniform(
                    kg, (), dtype=jnp.float32,
                    minval=float(np.finfo(np.float32).tiny), maxval=1.0,
                )
            )
            offset = int(np.floor(np.float32(np.log(np.float32(u))) / np.float32(log1mp)))
            offset = max(offset, 1)
            gap_start = last_end + offset
            if gap_start >= L - 1:
                break
            glen = int(np.int32(
                float(jax.random.normal(kn, (), dtype=jnp.float32)) * HOLE_SCALE + HOLE_LOC
            ))
            glen = max(glen, MIN_HOLE)
            gap_end = min(gap_start + glen, L)
            mask[gap_start:gap_end] = True
            last_end = gap_end
            if gap_end >= L:
                break
    _cache["mask"] = mask
    return mask


K = 6                    # SBUF tile slots in flight
T = RB * NF              # 16 tiles per core


def _build_nc():
    """Hand-scheduled raw-Bass pipeline.

    The walrus codegen here encodes at most ONE sync wait per instruction, so
    the schedule is built so every instruction needs at most one:
      - SP (sync/HWDGE)  issues loads, gated on store progress (slot reuse)
      - DVE              multiplies in place, gated on load progress only —
                         the WAR hazard vs the slot's previous store is covered
                         transitively because the load was issue-gated on it
      - ACT (scalar/HWDGE) issues stores, gated on multiply progress
    All loads count one semaphore, all stores another, multiplies a third, so
    every gate is a single cumulative wait.
    """
    if "nc" in _cache:
        return _cache["nc"]
    import concourse.bass as bass
    import concourse.mybir as mybir

    nc = bass.Bass()
    x = nc.declare_dram_parameter("x", [ROWS, L], mybir.dt.float32, isOutput=False)
    s = nc.declare_dram_parameter("s", [1, L], mybir.dt.float32, isOutput=False)
    out = nc.declare_dram_parameter("out", [ROWS, L], mybir.dt.float32, isOutput=True)

    f32 = mybir.dt.float32
    with (
        nc.sbuf_tensor([P, L], f32) as scale_sb,
        nc.sbuf_tensor([P, 1], f32) as probe,
        nc.sbuf_tensor([P, K, FCH], f32) as tiles,
        nc.semaphore("bcast_sem") as bcast_sem,
        nc.semaphore("load_sem") as load_sem,
        nc.semaphore("store_sem") as store_sem,
        nc.semaphore("vec_sem") as vec_sem,
        nc.Block() as block,
    ):
        def chunk(t):
            i, j = divmod(t, NF)
            rows = slice(i * P, (i + 1) * P)
            cols = slice(j * FCH, (j + 1) * FCH)
            return rows, cols, j

        @block.gpsimd
        def _(g):
            # Replicate the [1, L] scale vector into all 128 partitions with a
            # partition-stride-0 DMA read (one-time ~5 MB HBM read).
            s_bcast = bass.AP(tensor=s[0, :].tensor, offset=s[0, :].offset,
                              ap=[[0, P]] + list(s[0, :].ap))
            g.dma_start(out=scale_sb[:, :], in_=s_bcast).then_inc(bcast_sem, 16)

        @block.sync
        def _(sp):
            for t in range(T):
                if t >= K:
                    sp.wait_ge(store_sem, 16 * (t - K + 1))
                rows, cols, _ = chunk(t)
                sp.dma_start(
                    out=tiles[:, t % K, :], in_=x[rows, cols]
                ).then_inc(load_sem, 16)

        @block.vector
        def _(v):
            # Absorb the broadcast wait into a throwaway copy so no later
            # instruction ever carries two fused waits.
            v.wait_ge(bcast_sem, 16)
            v.tensor_copy(out=probe[:, :], in_=scale_sb[:, 0:1])
            for t in range(T):
                v.wait_ge(load_sem, 16 * (t + 1))
                _, cols, _ = chunk(t)
                v.tensor_mul(
                    out=tiles[:, t % K, :],
                    in0=tiles[:, t % K, :],
                    in1=scale_sb[:, cols],
                )
                # DVE pipe-drain fence: this op can't issue until the mul's
                # 8-slice pipe has emptied, so its inc proves the mul's SBUF
                # writes are visible to the store DMA (the mul's own then_inc
                # can fire before the write pipe drains).
                v.tensor_copy(out=probe[:, :], in_=probe[:, :]).then_inc(vec_sem, 1)

        @block.scalar
        def _(a):
            for t in range(T):
                a.wait_ge(vec_sem, t + 1)
                rows, cols, _ = chunk(t)
                a.dma_start(
                    out=out[rows, cols], in_=tiles[:, t % K, :]
                ).then_inc(store_sem, 16)

    _cache["nc"] = nc
    return nc


def kernel(x: np.ndarray, scaling_factor: np.ndarray, **run_kwargs) -> np.ndarray:
    from concourse.bass_utils import run_bass_kernel_spmd

    mask = _chunk_mask_np()
    scale_vec = ((1.0 - mask.astype(np.float32))
                 * np.float32(scaling_factor.reshape(-1)[0])).astype(np.float32)
    scale_vec = np.ascontiguousarray(scale_vec.reshape(1, L))

    x = np.ascontiguousarray(np.asarray(x, dtype=np.float32))
    nc = _build_nc()
    in_maps = [
        {"x": x[c * ROWS:(c + 1) * ROWS], "s": scale_vec} for c in range(N_CORES)
    ]
    res = run_bass_kernel_spmd(nc, in_maps, core_ids=list(range(N_CORES)), **run_kwargs)
    out = np.concatenate([r["out"] for r in res.results], axis=0)
    if "exec_time_ns" in dir(res):
        _cache["last_result"] = res
    return out
